# revision 17
# baseline (speedup 1.0000x reference)
"""DeepSeekMoE (E=8, top-2) forward as a Trainium2 Bass kernel.

Strategy: tensor parallelism over the expert FFN hidden dim F (not expert
parallelism).  Every core processes ALL 16384 (token, expert) pairs, sorted
by expert id, but owns only a 512-wide slice of F for every expert:

  core c holds W1[:, :, 512c:512(c+1)]  and  W2[:, 512c:512(c+1), :].

Why: per-core work is then exactly total/8 regardless of routing balance
(expert parallelism needs a common padded capacity: 2304 vs 2048 here, a
12.5%% PE-time tax), and the expert-segment boundaries in the pair list are
identical on every core, so the single SPMD program can switch weights at
exact token offsets with zero padding.

Host does routing, dispatch, and the whole gate combine in fp32 (top-2
softmax weights are applied on the host while summing the 8 partial-F
outputs), so the device runs nothing but the two grouped GEMMs + silu.

Device layout per chunk of T=512 pairs (weights stationary, lhsT):
  M1: ps1[f_tile 128, T]  += W1s[d, f_tile].T @ x[d, T]    (d = 8 K-tiles)
      h[f_tile, T] = silu(ps1 + b1)                         (bf16)
  M2: ps2[d_tile 128, T]  += W2s[k, d_tile].T @ h[k, T]    (k = 4 K-tiles)
      y[d_tile, T] = cast_bf16(ps2)          -> output in [D, pairs] layout
A chunk that straddles an expert boundary splits each matmul's moving
free-range at the boundary (all cores share the same boundary).

All matmuls run in bf16 (fp32 PSUM accumulation).
"""

import os
import sys

import numpy as np

sys.path.insert(0, "/opt/trn_rl_repo")

import ml_dtypes  # noqa: E402

import concourse.bass as bass  # noqa: E402
import concourse.tile as tile  # noqa: E402
from concourse import mybir  # noqa: E402
from concourse.bass import ds, ts  # noqa: E402
from concourse.bass_utils import run_bass_kernel_spmd  # noqa: E402

NUM_EXPERTS = 8
TOP_K = 2
D = 1024
F = 4096
FS = F // NUM_EXPERTS  # 512: per-core F slice
CHUNK = 512  # pairs per device-side pipeline chunk
N_PAIRS = 2 * 8192  # T * TOP_K, fixed by the problem shape
BF16 = mybir.dt.bfloat16
F32 = mybir.dt.float32

_AF = mybir.ActivationFunctionType
_ALU = mybir.AluOpType


def _legalize_waits(nc: bass.Bass, max_waits: int = 1) -> int:
    """This container's walrus build can encode at most ONE semaphore wait
    per instruction ("Too many sync wait commands" otherwise — even the
    repo's own Tile kernels trip it). Hoist extra waits onto same-engine
    NoOps inserted immediately before the offending instruction."""
    n_fix = 0
    for f in nc.m.functions:
        for blk in f.blocks:
            idx = 0
            while idx < len(blk.instructions):
                inst = blk.instructions[idx]
                si = inst.sync_info
                if (
                    si is not None
                    and si.on_wait
                    and len(si.on_wait) > max_waits
                    and type(inst).__name__ != "InstNoOp"
                ):
                    waits = list(si.on_wait)
                    keep, extra = waits[-max_waits:], waits[:-max_waits]
                    for j, w in enumerate(extra):
                        nop = mybir.InstNoOp(
                            name=f"LGW-{nc.next_id()}", ins=[], outs=[]
                        )
                        nop.engine = inst.engine
                        nop.sync_info = mybir.SyncInfo(on_wait=[w], on_update=[])
                        nc.register_instruction(nop)
                        blk.instructions.insert(idx + j, nop)
                    inst.sync_info = mybir.SyncInfo(
                        on_wait=keep, on_update=list(si.on_update)
                    )
                    idx += len(extra) + 1
                    n_fix += 1
                else:
                    idx += 1
    return n_fix


def _chunk_segs(bounds: list[int], c0: int, c1: int) -> list[tuple[int, int, int]]:
    """Expert segments intersecting pair range [c0, c1): (r0, r1, e) with
    r relative to c0."""
    segs = []
    for e in range(NUM_EXPERTS):
        b0, b1 = bounds[e], bounds[e + 1]
        lo, hi = max(b0, c0), min(b1, c1)
        if lo < hi:
            segs.append((lo - c0, hi - c0, e))
    return segs


def _build_program(bounds: list[int], use_b1: bool) -> bass.Bass:
    """Trace the single SPMD program run by all 8 cores.

    bounds: 9 cumulative expert boundaries over the sorted pair list.
    """
    n_chunks = N_PAIRS // CHUNK
    n_d = D // 128  # 8 contraction tiles for matmul1
    n_f = FS // 128  # 4 F-slice tiles
    assert bounds[-1] == N_PAIRS

    nc = bass.Bass(debug=False)
    # All HBM tensors are host-prepped in device layout (partition-major,
    # chunk-contiguous) so every DMA is one long contiguous run per
    # partition instead of 8x 1KB strided runs.
    xT_d = nc.declare_dram_parameter(
        "xT", [128, n_chunks, n_d, CHUNK], BF16, isOutput=False
    )
    w1_d = nc.declare_dram_parameter(
        "w1", [128, NUM_EXPERTS, n_d, FS], BF16, isOutput=False
    )
    w2_d = nc.declare_dram_parameter(
        "w2", [128, NUM_EXPERTS, n_f, D], BF16, isOutput=False
    )
    if use_b1:
        b1_d = nc.declare_dram_parameter(
            "b1", [128, NUM_EXPERTS, n_f], F32, isOutput=False
        )
    y_d = nc.declare_dram_parameter(
        "y", [128, n_chunks, n_d, CHUNK], BF16, isOutput=True
    )

    with tile.TileContext(nc) as tc:
        with (
            tc.tile_pool(name="consts", bufs=1) as consts,
            tc.tile_pool(name="xin", bufs=3) as xin,
            tc.tile_pool(name="hbuf", bufs=2) as hbuf,
            tc.tile_pool(name="ybuf", bufs=2) as ybuf,
            tc.tile_pool(name="ps1p", bufs=4, space="PSUM") as ps1p,
            tc.tile_pool(name="ps2p", bufs=4, space="PSUM") as ps2p,
        ):
            # ---- HAM warm-up: matmuls on memset data run while the first
            # x/W DMAs are in flight, so real matmuls start at 2.4 GHz.
            warm_sb = consts.tile([128, CHUNK], BF16)
            nc.vector.memset(warm_sb[:], 1.0)
            for _ in range(8):
                ps_w = ps1p.tile([128, CHUNK], F32, tag="ps1")
                nc.tensor.matmul(
                    ps_w[:], warm_sb[:, 0:128], warm_sb[:], start=True, stop=True
                )

            # ---- resident weights ----
            w1_sb = consts.tile([128, NUM_EXPERTS, n_d, FS], BF16)
            w2_sb = consts.tile([128, NUM_EXPERTS, n_f, D], BF16)
            if use_b1:
                b1_sb = consts.tile([128, NUM_EXPERTS, n_f], F32)
                nc.sync.dma_start(b1_sb[:], b1_d[:])

            # ---- main pipeline over pair chunks ----
            chunk_list = [(i * CHUNK, CHUNK) for i in range(n_chunks)]
            for c, (c0, sz) in enumerate(chunk_list):
                segs = _chunk_segs(bounds, c0, c0 + sz)

                x_c = xin.tile([128, n_d, CHUNK], BF16, tag="x")
                nc.sync.dma_start(x_c[:, :, 0:sz], xT_d[:, c, :, 0:sz])
                if c == 0:
                    # expert 0's weights, w1 in f-tile pieces so chunk 0's
                    # M1 f-loop can start as soon as the first piece lands
                    for f in range(n_f):
                        nc.sync.dma_start(
                            w1_sb[:, 0, :, ts(f, 128)], w1_d[:, 0, :, ts(f, 128)]
                        )
                    nc.sync.dma_start(w2_sb[:, 0, 0:2], w2_d[:, 0, 0:2])
                    nc.sync.dma_start(w2_sb[:, 0, 2:4], w2_d[:, 0, 2:4])
                elif c <= 2 * (NUM_EXPERTS - 1):
                    # stream expert e's weights in 0.5MB halves over chunks
                    # 2e-1 (w1) and 2e (w2): ~1MB/chunk extra DMA, done well
                    # before first use at chunk >= 3.77*e
                    e = (c + 1) // 2
                    if c % 2 == 1:
                        nc.sync.dma_start(w1_sb[:, e, 0:4], w1_d[:, e, 0:4])
                        nc.sync.dma_start(w1_sb[:, e, 4:8], w1_d[:, e, 4:8])
                    else:
                        nc.sync.dma_start(w2_sb[:, e, 0:2], w2_d[:, e, 0:2])
                        nc.sync.dma_start(w2_sb[:, e, 2:4], w2_d[:, e, 2:4])

                # matmul1 + silu: h tiles [128(F-slice), CHUNK].  Each expert
                # segment gets its OWN PSUM tile: interleaving two accumulation
                # groups in one bank corrupts the earlier group's region.
                h_c = hbuf.tile([128, n_f, CHUNK], BF16, tag="h")
                for f in range(n_f):
                    for r0, r1, e in segs:
                        w = r1 - r0
                        ps1 = ps1p.tile([128, CHUNK], F32, tag="ps1")
                        for d in range(n_d):
                            nc.tensor.matmul(
                                ps1[:, 0:w],
                                w1_sb[:, e, d, ts(f, 128)],
                                x_c[:, d, r0:r1],
                                start=(d == 0),
                                stop=(d == n_d - 1),
                            )
                        if use_b1:
                            nc.scalar.activation(
                                h_c[:, f, r0:r1], ps1[:, 0:w], _AF.Silu,
                                bias=b1_sb[:, e, f : f + 1],
                            )
                        else:
                            nc.scalar.activation(
                                h_c[:, f, r0:r1], ps1[:, 0:w], _AF.Silu
                            )
                # matmul2: yT tiles [128(D), CHUNK]
                y_c = ybuf.tile([128, n_d, CHUNK], BF16, tag="y")
                for n in range(n_d):
                    for r0, r1, e in segs:
                        w = r1 - r0
                        ps2 = ps2p.tile([128, CHUNK], F32, tag="ps2")
                        for k in range(n_f):
                            nc.tensor.matmul(
                                ps2[:, 0:w],
                                w2_sb[:, e, k, ts(n, 128)],
                                h_c[:, k, r0:r1],
                                start=(k == 0),
                                stop=(k == n_f - 1),
                            )
                        # f32->bf16 cast; alternate engines (gpsimd can't read PSUM)
                        if n % 2 == 0:
                            nc.vector.tensor_scalar_mul(
                                y_c[:, n, r0:r1], ps2[:, 0:w], 1.0
                            )
                        else:
                            nc.scalar.copy(y_c[:, n, r0:r1], ps2[:, 0:w])
                    if n == 3:
                        # trailing y DMA in shrinking pieces: the final
                        # transfer after the last cast is only 2 n-tiles
                        nc.sync.dma_start(
                            y_d[:, c, 0:4, 0:sz], y_c[:, 0:4, 0:sz]
                        )
                    elif n == 5:
                        nc.sync.dma_start(
                            y_d[:, c, 4:6, 0:sz], y_c[:, 4:6, 0:sz]
                        )
                if c == n_chunks - 1:
                    # last chunk: per-tile trailing DMAs shorten the drain
                    nc.sync.dma_start(y_d[:, c, 6:7, 0:sz], y_c[:, 6:7, 0:sz])
                    nc.sync.dma_start(y_d[:, c, 7:8, 0:sz], y_c[:, 7:8, 0:sz])
                else:
                    nc.sync.dma_start(y_d[:, c, 6:8, 0:sz], y_c[:, 6:8, 0:sz])

    _legalize_waits(nc)
    return nc


def _enable_tracing_shims():
    """Profiling-only (MOE_KERNEL_TRACE=1): install the NTFF profile hook
    that the boot skips when antenv.axon_hooks is missing, and stub out the
    artifact upload (no network in this sandbox)."""
    import types

    try:
        import antenv.axon_hooks  # noqa: F401
    except ImportError:
        try:
            import antenv
            from trn_agent_boot.trn_boot import _ntff_profile_via_ctypes

            hook = _ntff_profile_via_ctypes("/opt/axon/libaxon_pjrt.so")
            mod = types.ModuleType("antenv.axon_hooks")
            mod._hook = hook
            mod.get_axon_ntff_profile_hook = lambda: mod._hook
            mod.set_axon_ntff_profile_hook = lambda h: setattr(mod, "_hook", h)
            sys.modules["antenv.axon_hooks"] = mod
            antenv.axon_hooks = mod
        except Exception as e:  # pragma: no cover
            print(f"NTFF hook install failed: {e}", file=sys.stderr)

    import concourse.bass_utils as _bu

    _bu.upload_artifacts = lambda tmpdir: f"local:{tmpdir}"


def kernel(**inputs) -> np.ndarray:
    x = np.asarray(inputs["x"], dtype=np.float32)
    gate_w = np.asarray(inputs["gate_w"], dtype=np.float32)
    gate_b = np.asarray(inputs["gate_b"], dtype=np.float32)
    W1 = np.asarray(inputs["W1"], dtype=np.float32)
    b1 = np.asarray(inputs["b1"], dtype=np.float32)
    W2 = np.asarray(inputs["W2"], dtype=np.float32)
    b2 = np.asarray(inputs["b2"], dtype=np.float32)

    B, S, D_ = x.shape
    T = B * S
    xf = x.reshape(T, D_)
    assert TOP_K * T == N_PAIRS and D_ == D

    # ---- host: routing + top-2 softmax combine weights (all fp32) ----
    logits = xf @ gate_w + gate_b  # [T, E]
    top2 = np.argpartition(-logits, TOP_K - 1, axis=1)[:, :TOP_K]
    lv = np.take_along_axis(logits, top2, 1)
    ex = np.exp(lv - lv.max(axis=1, keepdims=True))
    tw = ex / ex.sum(axis=1, keepdims=True)  # [T, 2]

    sel = np.zeros((T, NUM_EXPERTS), dtype=bool)
    sel[np.arange(T)[:, None], top2] = True
    idx_per_e = [np.nonzero(sel[:, e])[0] for e in range(NUM_EXPERTS)]
    w_per_e = []
    for e in range(NUM_EXPERTS):
        idx = idx_per_e[e]
        w_per_e.append(np.where(top2[idx, 0] == e, tw[idx, 0], tw[idx, 1]))
    counts = [len(i) for i in idx_per_e]
    bounds = [0]
    for e in range(NUM_EXPERTS):
        bounds.append(bounds[-1] + counts[e])

    # ---- dispatch: expert-sorted pair list, shared by all cores ----
    # device layouts are partition-major so every DMA run is contiguous:
    #   xT[p, c, d, t] = x_pairs[c*CHUNK+t, d*128+p]
    #   w1[p, e, d, f] = W1[e, d*128+p, fsl][f];  w2[p, e, k, n] = W2[e, k*128+p+fsl0, n]
    pair_tok = np.concatenate(idx_per_e)
    n_chunks = N_PAIRS // CHUNK
    xg = xf[pair_tok].astype(ml_dtypes.bfloat16)  # [N_PAIRS, D]
    xT = np.ascontiguousarray(
        xg.reshape(n_chunks, CHUNK, D // 128, 128).transpose(3, 0, 2, 1)
    )

    use_b1 = bool(np.any(b1 != 0.0))
    in_maps = []
    for c in range(NUM_EXPERTS):
        fsl = slice(c * FS, (c + 1) * FS)
        w1s = W1[:, :, fsl].astype(ml_dtypes.bfloat16)  # [E, D, FS]
        w2s = W2[:, fsl, :].astype(ml_dtypes.bfloat16)  # [E, FS, D]
        m = {
            "xT": xT,
            "w1": np.ascontiguousarray(
                w1s.reshape(NUM_EXPERTS, D // 128, 128, FS).transpose(2, 0, 1, 3)
            ),
            "w2": np.ascontiguousarray(
                w2s.reshape(NUM_EXPERTS, FS // 128, 128, D).transpose(2, 0, 1, 3)
            ),
        }
        if use_b1:
            m["b1"] = np.ascontiguousarray(
                b1[:, fsl].reshape(NUM_EXPERTS, FS // 128, 128).transpose(2, 0, 1)
            )
        in_maps.append(m)

    nc = _build_program(bounds, use_b1)
    trace = bool(int(os.environ.get("MOE_KERNEL_TRACE", "0")))
    if trace:
        _enable_tracing_shims()
    res = run_bass_kernel_spmd(nc, in_maps, list(range(NUM_EXPERTS)), trace=trace)
    if trace:
        kernel.last_results = res

    # ---- combine: sum partial-F outputs, apply gate weight, un-dispatch ----
    acc = np.zeros((128, n_chunks, D // 128, CHUNK), dtype=np.float32)
    for c in range(NUM_EXPERTS):
        acc += res.results[c]["y"].astype(np.float32)
    # [p, c, n, t] -> [n*128+p, c*CHUNK+t] -> pairs on rows
    y_pairs = np.ascontiguousarray(
        acc.transpose(2, 0, 1, 3).reshape(D, N_PAIRS).T
    )  # [N_PAIRS, D]

    out = np.zeros((T, D_), dtype=np.float32)
    for e in range(NUM_EXPERTS):
        seg = y_pairs[bounds[e] : bounds[e + 1]]
        if np.any(b2[e] != 0.0):
            seg = seg + b2[e]
        out[idx_per_e[e]] += w_per_e[e][:, None] * seg
    return out.reshape(B, S, D_)


# revision 18
# speedup vs baseline: 1.0032x; 1.0032x over previous
"""DeepSeekMoE (E=8, top-2) forward as a Trainium2 Bass kernel.

Strategy: tensor parallelism over the expert FFN hidden dim F (not expert
parallelism).  Every core processes ALL 16384 (token, expert) pairs, sorted
by expert id, but owns only a 512-wide slice of F for every expert:

  core c holds W1[:, :, 512c:512(c+1)]  and  W2[:, 512c:512(c+1), :].

Why: per-core work is then exactly total/8 regardless of routing balance
(expert parallelism needs a common padded capacity: 2304 vs 2048 here, a
12.5%% PE-time tax), and the expert-segment boundaries in the pair list are
identical on every core, so the single SPMD program can switch weights at
exact token offsets with zero padding.

Host does routing, dispatch, and the whole gate combine in fp32 (top-2
softmax weights are applied on the host while summing the 8 partial-F
outputs), so the device runs nothing but the two grouped GEMMs + silu.

Device layout per chunk of T=512 pairs (weights stationary, lhsT):
  M1: ps1[f_tile 128, T]  += W1s[d, f_tile].T @ x[d, T]    (d = 8 K-tiles)
      h[f_tile, T] = silu(ps1 + b1)                         (bf16)
  M2: ps2[d_tile 128, T]  += W2s[k, d_tile].T @ h[k, T]    (k = 4 K-tiles)
      y[d_tile, T] = cast_bf16(ps2)          -> output in [D, pairs] layout
A chunk that straddles an expert boundary splits each matmul's moving
free-range at the boundary (all cores share the same boundary).

All matmuls run in bf16 (fp32 PSUM accumulation).
"""

import os
import sys

import numpy as np

sys.path.insert(0, "/opt/trn_rl_repo")

import ml_dtypes  # noqa: E402

import concourse.bass as bass  # noqa: E402
import concourse.tile as tile  # noqa: E402
from concourse import mybir  # noqa: E402
from concourse.bass import ds, ts  # noqa: E402
from concourse.bass_utils import run_bass_kernel_spmd  # noqa: E402

NUM_EXPERTS = 8
TOP_K = 2
D = 1024
F = 4096
FS = F // NUM_EXPERTS  # 512: per-core F slice
CHUNK = 512  # pairs per device-side pipeline chunk
N_PAIRS = 2 * 8192  # T * TOP_K, fixed by the problem shape
BF16 = mybir.dt.bfloat16
F32 = mybir.dt.float32

_AF = mybir.ActivationFunctionType
_ALU = mybir.AluOpType


def _legalize_waits(nc: bass.Bass, max_waits: int = 1) -> int:
    """This container's walrus build can encode at most ONE semaphore wait
    per instruction ("Too many sync wait commands" otherwise — even the
    repo's own Tile kernels trip it). Hoist extra waits onto same-engine
    NoOps inserted immediately before the offending instruction."""
    n_fix = 0
    for f in nc.m.functions:
        for blk in f.blocks:
            idx = 0
            while idx < len(blk.instructions):
                inst = blk.instructions[idx]
                si = inst.sync_info
                if (
                    si is not None
                    and si.on_wait
                    and len(si.on_wait) > max_waits
                    and type(inst).__name__ != "InstNoOp"
                ):
                    waits = list(si.on_wait)
                    keep, extra = waits[-max_waits:], waits[:-max_waits]
                    for j, w in enumerate(extra):
                        nop = mybir.InstNoOp(
                            name=f"LGW-{nc.next_id()}", ins=[], outs=[]
                        )
                        nop.engine = inst.engine
                        nop.sync_info = mybir.SyncInfo(on_wait=[w], on_update=[])
                        nc.register_instruction(nop)
                        blk.instructions.insert(idx + j, nop)
                    inst.sync_info = mybir.SyncInfo(
                        on_wait=keep, on_update=list(si.on_update)
                    )
                    idx += len(extra) + 1
                    n_fix += 1
                else:
                    idx += 1
    return n_fix


def _chunk_segs(bounds: list[int], c0: int, c1: int) -> list[tuple[int, int, int]]:
    """Expert segments intersecting pair range [c0, c1): (r0, r1, e) with
    r relative to c0."""
    segs = []
    for e in range(NUM_EXPERTS):
        b0, b1 = bounds[e], bounds[e + 1]
        lo, hi = max(b0, c0), min(b1, c1)
        if lo < hi:
            segs.append((lo - c0, hi - c0, e))
    return segs


def _build_program(bounds: list[int], use_b1: bool) -> bass.Bass:
    """Trace the single SPMD program run by all 8 cores.

    bounds: 9 cumulative expert boundaries over the sorted pair list.
    """
    n_chunks = N_PAIRS // CHUNK
    n_d = D // 128  # 8 contraction tiles for matmul1
    n_f = FS // 128  # 4 F-slice tiles
    assert bounds[-1] == N_PAIRS

    nc = bass.Bass(debug=False)
    # All HBM tensors are host-prepped in device layout (partition-major,
    # chunk-contiguous) so every DMA is one long contiguous run per
    # partition instead of 8x 1KB strided runs.
    xT_d = nc.declare_dram_parameter(
        "xT", [128, n_chunks, n_d, CHUNK], BF16, isOutput=False
    )
    w1_d = nc.declare_dram_parameter(
        "w1", [128, NUM_EXPERTS, n_d, FS], BF16, isOutput=False
    )
    w2_d = nc.declare_dram_parameter(
        "w2", [128, NUM_EXPERTS, n_f, D], BF16, isOutput=False
    )
    if use_b1:
        b1_d = nc.declare_dram_parameter(
            "b1", [128, NUM_EXPERTS, n_f], F32, isOutput=False
        )
    y_d = nc.declare_dram_parameter(
        "y", [128, n_chunks, n_d, CHUNK], BF16, isOutput=True
    )

    with tile.TileContext(nc) as tc:
        with (
            tc.tile_pool(name="consts", bufs=1) as consts,
            tc.tile_pool(name="xin", bufs=3) as xin,
            tc.tile_pool(name="hbuf", bufs=2) as hbuf,
            tc.tile_pool(name="ybuf", bufs=2) as ybuf,
            tc.tile_pool(name="ps1p", bufs=4, space="PSUM") as ps1p,
            tc.tile_pool(name="ps2p", bufs=4, space="PSUM") as ps2p,
        ):
            # ---- HAM warm-up: matmuls on memset data run while the first
            # x/W DMAs are in flight, so real matmuls start at 2.4 GHz.
            warm_sb = consts.tile([128, CHUNK], BF16)
            nc.vector.memset(warm_sb[:], 1.0)
            for _ in range(10):
                ps_w = ps1p.tile([128, CHUNK], F32, tag="ps1")
                nc.tensor.matmul(
                    ps_w[:], warm_sb[:, 0:128], warm_sb[:], start=True, stop=True
                )

            # ---- resident weights ----
            w1_sb = consts.tile([128, NUM_EXPERTS, n_d, FS], BF16)
            w2_sb = consts.tile([128, NUM_EXPERTS, n_f, D], BF16)
            if use_b1:
                b1_sb = consts.tile([128, NUM_EXPERTS, n_f], F32)
                nc.sync.dma_start(b1_sb[:], b1_d[:])

            # ---- main pipeline over pair chunks ----
            chunk_list = [(i * CHUNK, CHUNK) for i in range(n_chunks)]
            for c, (c0, sz) in enumerate(chunk_list):
                segs = _chunk_segs(bounds, c0, c0 + sz)

                x_c = xin.tile([128, n_d, CHUNK], BF16, tag="x")
                nc.sync.dma_start(x_c[:, :, 0:sz], xT_d[:, c, :, 0:sz])
                if c == 0:
                    # expert 0's weights, w1 in f-tile pieces so chunk 0's
                    # M1 f-loop can start as soon as the first piece lands
                    for f in range(n_f):
                        nc.sync.dma_start(
                            w1_sb[:, 0, :, ts(f, 128)], w1_d[:, 0, :, ts(f, 128)]
                        )
                    nc.sync.dma_start(w2_sb[:, 0, 0:2], w2_d[:, 0, 0:2])
                    nc.sync.dma_start(w2_sb[:, 0, 2:4], w2_d[:, 0, 2:4])
                elif c <= 2 * (NUM_EXPERTS - 1):
                    # stream expert e's weights in 0.5MB halves over chunks
                    # 2e-1 (w1) and 2e (w2): ~1MB/chunk extra DMA, done well
                    # before first use at chunk >= 3.77*e
                    e = (c + 1) // 2
                    if c % 2 == 1:
                        nc.sync.dma_start(w1_sb[:, e, 0:4], w1_d[:, e, 0:4])
                        nc.sync.dma_start(w1_sb[:, e, 4:8], w1_d[:, e, 4:8])
                    else:
                        nc.sync.dma_start(w2_sb[:, e, 0:2], w2_d[:, e, 0:2])
                        nc.sync.dma_start(w2_sb[:, e, 2:4], w2_d[:, e, 2:4])

                # matmul1 + silu: h tiles [128(F-slice), CHUNK].  Each expert
                # segment gets its OWN PSUM tile: interleaving two accumulation
                # groups in one bank corrupts the earlier group's region.
                h_c = hbuf.tile([128, n_f, CHUNK], BF16, tag="h")
                for f in range(n_f):
                    for r0, r1, e in segs:
                        w = r1 - r0
                        ps1 = ps1p.tile([128, CHUNK], F32, tag="ps1")
                        for d in range(n_d):
                            nc.tensor.matmul(
                                ps1[:, 0:w],
                                w1_sb[:, e, d, ts(f, 128)],
                                x_c[:, d, r0:r1],
                                start=(d == 0),
                                stop=(d == n_d - 1),
                            )
                        if use_b1:
                            nc.scalar.activation(
                                h_c[:, f, r0:r1], ps1[:, 0:w], _AF.Silu,
                                bias=b1_sb[:, e, f : f + 1],
                            )
                        else:
                            nc.scalar.activation(
                                h_c[:, f, r0:r1], ps1[:, 0:w], _AF.Silu
                            )
                # matmul2: yT tiles [128(D), CHUNK]
                y_c = ybuf.tile([128, n_d, CHUNK], BF16, tag="y")
                for n in range(n_d):
                    for r0, r1, e in segs:
                        w = r1 - r0
                        ps2 = ps2p.tile([128, CHUNK], F32, tag="ps2")
                        for k in range(n_f):
                            nc.tensor.matmul(
                                ps2[:, 0:w],
                                w2_sb[:, e, k, ts(n, 128)],
                                h_c[:, k, r0:r1],
                                start=(k == 0),
                                stop=(k == n_f - 1),
                            )
                        # f32->bf16 cast; alternate engines (gpsimd can't read PSUM)
                        if n % 2 == 0:
                            nc.vector.tensor_scalar_mul(
                                y_c[:, n, r0:r1], ps2[:, 0:w], 1.0
                            )
                        else:
                            nc.scalar.copy(y_c[:, n, r0:r1], ps2[:, 0:w])
                    if n == 3:
                        # trailing y DMA in shrinking pieces: the final
                        # transfer after the last cast is only 2 n-tiles
                        nc.sync.dma_start(
                            y_d[:, c, 0:4, 0:sz], y_c[:, 0:4, 0:sz]
                        )
                    elif n == 5:
                        nc.sync.dma_start(
                            y_d[:, c, 4:6, 0:sz], y_c[:, 4:6, 0:sz]
                        )
                nc.sync.dma_start(y_d[:, c, 6:8, 0:sz], y_c[:, 6:8, 0:sz])

    _legalize_waits(nc)
    return nc


def _enable_tracing_shims():
    """Profiling-only (MOE_KERNEL_TRACE=1): install the NTFF profile hook
    that the boot skips when antenv.axon_hooks is missing, and stub out the
    artifact upload (no network in this sandbox)."""
    import types

    try:
        import antenv.axon_hooks  # noqa: F401
    except ImportError:
        try:
            import antenv
            from trn_agent_boot.trn_boot import _ntff_profile_via_ctypes

            hook = _ntff_profile_via_ctypes("/opt/axon/libaxon_pjrt.so")
            mod = types.ModuleType("antenv.axon_hooks")
            mod._hook = hook
            mod.get_axon_ntff_profile_hook = lambda: mod._hook
            mod.set_axon_ntff_profile_hook = lambda h: setattr(mod, "_hook", h)
            sys.modules["antenv.axon_hooks"] = mod
            antenv.axon_hooks = mod
        except Exception as e:  # pragma: no cover
            print(f"NTFF hook install failed: {e}", file=sys.stderr)

    import concourse.bass_utils as _bu

    _bu.upload_artifacts = lambda tmpdir: f"local:{tmpdir}"


def kernel(**inputs) -> np.ndarray:
    x = np.asarray(inputs["x"], dtype=np.float32)
    gate_w = np.asarray(inputs["gate_w"], dtype=np.float32)
    gate_b = np.asarray(inputs["gate_b"], dtype=np.float32)
    W1 = np.asarray(inputs["W1"], dtype=np.float32)
    b1 = np.asarray(inputs["b1"], dtype=np.float32)
    W2 = np.asarray(inputs["W2"], dtype=np.float32)
    b2 = np.asarray(inputs["b2"], dtype=np.float32)

    B, S, D_ = x.shape
    T = B * S
    xf = x.reshape(T, D_)
    assert TOP_K * T == N_PAIRS and D_ == D

    # ---- host: routing + top-2 softmax combine weights (all fp32) ----
    logits = xf @ gate_w + gate_b  # [T, E]
    top2 = np.argpartition(-logits, TOP_K - 1, axis=1)[:, :TOP_K]
    lv = np.take_along_axis(logits, top2, 1)
    ex = np.exp(lv - lv.max(axis=1, keepdims=True))
    tw = ex / ex.sum(axis=1, keepdims=True)  # [T, 2]

    sel = np.zeros((T, NUM_EXPERTS), dtype=bool)
    sel[np.arange(T)[:, None], top2] = True
    idx_per_e = [np.nonzero(sel[:, e])[0] for e in range(NUM_EXPERTS)]
    w_per_e = []
    for e in range(NUM_EXPERTS):
        idx = idx_per_e[e]
        w_per_e.append(np.where(top2[idx, 0] == e, tw[idx, 0], tw[idx, 1]))
    counts = [len(i) for i in idx_per_e]
    bounds = [0]
    for e in range(NUM_EXPERTS):
        bounds.append(bounds[-1] + counts[e])

    # ---- dispatch: expert-sorted pair list, shared by all cores ----
    # device layouts are partition-major so every DMA run is contiguous:
    #   xT[p, c, d, t] = x_pairs[c*CHUNK+t, d*128+p]
    #   w1[p, e, d, f] = W1[e, d*128+p, fsl][f];  w2[p, e, k, n] = W2[e, k*128+p+fsl0, n]
    pair_tok = np.concatenate(idx_per_e)
    n_chunks = N_PAIRS // CHUNK
    xg = xf[pair_tok].astype(ml_dtypes.bfloat16)  # [N_PAIRS, D]
    xT = np.ascontiguousarray(
        xg.reshape(n_chunks, CHUNK, D // 128, 128).transpose(3, 0, 2, 1)
    )

    use_b1 = bool(np.any(b1 != 0.0))
    in_maps = []
    for c in range(NUM_EXPERTS):
        fsl = slice(c * FS, (c + 1) * FS)
        w1s = W1[:, :, fsl].astype(ml_dtypes.bfloat16)  # [E, D, FS]
        w2s = W2[:, fsl, :].astype(ml_dtypes.bfloat16)  # [E, FS, D]
        m = {
            "xT": xT,
            "w1": np.ascontiguousarray(
                w1s.reshape(NUM_EXPERTS, D // 128, 128, FS).transpose(2, 0, 1, 3)
            ),
            "w2": np.ascontiguousarray(
                w2s.reshape(NUM_EXPERTS, FS // 128, 128, D).transpose(2, 0, 1, 3)
            ),
        }
        if use_b1:
            m["b1"] = np.ascontiguousarray(
                b1[:, fsl].reshape(NUM_EXPERTS, FS // 128, 128).transpose(2, 0, 1)
            )
        in_maps.append(m)

    nc = _build_program(bounds, use_b1)
    trace = bool(int(os.environ.get("MOE_KERNEL_TRACE", "0")))
    if trace:
        _enable_tracing_shims()
    res = run_bass_kernel_spmd(nc, in_maps, list(range(NUM_EXPERTS)), trace=trace)
    if trace:
        kernel.last_results = res

    # ---- combine: sum partial-F outputs, apply gate weight, un-dispatch ----
    acc = np.zeros((128, n_chunks, D // 128, CHUNK), dtype=np.float32)
    for c in range(NUM_EXPERTS):
        acc += res.results[c]["y"].astype(np.float32)
    # [p, c, n, t] -> [n*128+p, c*CHUNK+t] -> pairs on rows
    y_pairs = np.ascontiguousarray(
        acc.transpose(2, 0, 1, 3).reshape(D, N_PAIRS).T
    )  # [N_PAIRS, D]

    out = np.zeros((T, D_), dtype=np.float32)
    for e in range(NUM_EXPERTS):
        seg = y_pairs[bounds[e] : bounds[e + 1]]
        if np.any(b2[e] != 0.0):
            seg = seg + b2[e]
        out[idx_per_e[e]] += w_per_e[e][:, None] * seg
    return out.reshape(B, S, D_)
